# revision 9
# baseline (speedup 1.0000x reference)
"""Multi-scale voxel feature lookup + per-level projector MLP on 8 TRN2 cores.

Strategy: channel-shard the 5 feature volumes 8 ways (16 channels/core).
Host pre-transposes each level's shard to channel-last rows (4*S^3, 16) so a
point lookup is one contiguous 64B row per (level, sample). On device each
core computes flat voxel indices from p (int32 vector ops), performs 5
indirect-DMA row gathers, AllGathers the (5,4,16) feature shards (1.25KB/core)
across the chip, then every core runs the full 5-level MLP
(conv1x1 -> train-mode BN -> ReLU -> conv1x1) redundantly and writes an
identical (5,4,64) output; the host returns core 0's copy.

Note: b1 cancels exactly in train-mode BN (h - mean(h)), so it is unused.
"""

import numpy as np

import concourse.bass as bass
import concourse.tile as tile
from concourse import bacc, mybir
from concourse.bass_utils import run_bass_kernel_spmd

N_CORES = 8
BS = 4
C = 128
CS = C // N_CORES  # 16 channels per core
HID = 256
OUT = 64
SIDES = [64, 32, 16, 8, 4]
NLVL = 5
EPS = 1e-5

F32 = mybir.dt.float32
I32 = mybir.dt.int32


def _index_consts() -> np.ndarray:
    """(4, 35) int32: [shift amounts (15) | voxel weights (15) | batch offsets (5)].

    Column layout: for t in 0..2, level l: shifts[l*3+t] = l+1,
    wmul[l*3+t] = [S^2, S, 1][t]; boff[l] = b * S^3 (per-partition b).
    """
    icst = np.zeros((BS, 35), dtype=np.int32)
    for l, s in enumerate(SIDES):
        icst[:, l * 3 + 0] = l + 1
        icst[:, l * 3 + 1] = l + 1
        icst[:, l * 3 + 2] = l + 1
        icst[:, 15 + l * 3 + 0] = s * s
        icst[:, 15 + l * 3 + 1] = s
        icst[:, 15 + l * 3 + 2] = 1
        icst[:, 30 + l] = np.arange(BS) * s**3
    return icst


def build_program():
    nc = bacc.Bacc("TRN2", target_bir_lowering=False, debug=False, num_devices=N_CORES)

    xs = [
        nc.dram_tensor(f"x{l}s", [BS * s**3, CS], F32, kind="ExternalInput").ap()
        for l, s in enumerate(SIDES)
    ]
    p32 = nc.dram_tensor("p32", [BS, 3], I32, kind="ExternalInput").ap()
    w1t = nc.dram_tensor("w1t", [NLVL, C, HID], F32, kind="ExternalInput").ap()
    w2t = nc.dram_tensor("w2t", [NLVL, HID, OUT], F32, kind="ExternalInput").ap()
    gam = nc.dram_tensor("gam", [NLVL, HID], F32, kind="ExternalInput").ap()
    bet = nc.dram_tensor("bet", [NLVL, HID], F32, kind="ExternalInput").ap()
    b2r = nc.dram_tensor("b2r", [BS, NLVL * OUT], F32, kind="ExternalInput").ap()
    out = nc.dram_tensor("out", [NLVL, BS, OUT], F32, kind="ExternalOutput").ap()

    icst = nc.inline_tensor(_index_consts(), name="icst").ap()

    cc_in = nc.dram_tensor("cc_in", [NLVL * BS, CS], F32).ap()
    cc_out = nc.dram_tensor("cc_out", [N_CORES * NLVL * BS, CS], F32, addr_space="Shared").ap()

    NG = 2 * NLVL  # (half, level) groups; hidden split into 2 partition halves

    with tile.TileContext(nc) as tc:
        with (
            tc.tile_pool(name="sbuf", bufs=1) as sp,
            tc.tile_pool(name="psum", bufs=1, space="PSUM") as pp,
        ):
            # ---- weight preloads (independent of the gather path) ----
            w1sb = sp.tile([C, NLVL * HID], F32, tag="w1sb")
            nc.sync.dma_start(
                w1sb[:].rearrange("k (l m) -> k l m", l=NLVL),
                w1t.rearrange("l k m -> k l m"),
            )
            w2sb = sp.tile([C, NG * OUT], F32, tag="w2sb")
            nc.sync.dma_start(
                w2sb[:].rearrange("j (l h o) -> j l h o", h=2, l=NLVL),
                w2t.rearrange("l (h j) o -> j l h o", h=2),
            )
            gsb = sp.tile([C, NG], F32, tag="gsb")
            nc.sync.dma_start(
                gsb[:].rearrange("j (l h) -> j l h", h=2),
                gam.rearrange("l (h j) -> j l h", h=2),
            )
            bsb = sp.tile([C, NG], F32, tag="bsb")
            nc.sync.dma_start(
                bsb[:].rearrange("j (l h) -> j l h", h=2),
                bet.rearrange("l (h j) -> j l h", h=2),
            )
            b2sb = sp.tile([BS, NLVL * OUT], F32, tag="b2sb")
            nc.sync.dma_start(b2sb[:], b2r)

            # ---- index math: idx[b, l] = sum_t (p[b,t] >> (l+1)) * w[l,t] + b*S^3
            pi = sp.tile([BS, 3], I32, tag="pi")
            nc.sync.dma_start(pi[:], p32)
            ic = sp.tile([BS, 35], I32, tag="ic")
            nc.sync.dma_start(ic[:], icst)

            q = sp.tile([BS, 15], I32, tag="q")
            pib = pi[:].rearrange("b (l t) -> b l t", l=1).to_broadcast([BS, NLVL, 3])
            nc.vector.tensor_tensor(
                out=q[:].rearrange("b (l t) -> b l t", l=NLVL),
                in0=pib,
                in1=ic[:, 0:15].rearrange("b (l t) -> b l t", l=NLVL),
                op=mybir.AluOpType.logical_shift_right,
            )
            qw = sp.tile([BS, 15], I32, tag="qw")
            nc.vector.tensor_tensor(
                out=qw[:], in0=q[:], in1=ic[:, 15:30], op=mybir.AluOpType.mult
            )
            idx = sp.tile([BS, NLVL], I32, tag="idx")
            with nc.allow_low_precision(reason="exact int32 index sums"):
                nc.vector.reduce_sum(
                    out=idx[:].rearrange("b (l o) -> b l o", o=1),
                    in_=qw[:].rearrange("b (l t) -> b l t", l=NLVL),
                    axis=mybir.AxisListType.X,
                )
            idx2 = sp.tile([BS, NLVL], I32, tag="idx2")
            nc.vector.tensor_tensor(
                out=idx2[:], in0=idx[:], in1=ic[:, 30:35], op=mybir.AluOpType.add
            )

            # ---- indirect row gathers: feats[b, l*16+j] = x_l[row idx2[b,l], j]
            feats = sp.tile([BS, NLVL * CS], F32, tag="feats")
            for l in range(NLVL):
                nc.gpsimd.indirect_dma_start(
                    out=feats[:, l * CS : (l + 1) * CS],
                    out_offset=None,
                    in_=xs[l],
                    in_offset=bass.IndirectOffsetOnAxis(ap=idx2[:, l : l + 1], axis=0),
                )

            # ---- AllGather the per-core channel shards
            nc.sync.dma_start(
                cc_in.rearrange("(l b) j -> b l j", l=NLVL),
                feats[:].rearrange("b (l j) -> b l j", l=NLVL),
            )
            nc.gpsimd.collective_compute(
                "AllGather",
                mybir.AluOpType.bypass,
                replica_groups=[list(range(N_CORES))],
                ins=[cc_in],
                outs=[cc_out],
            )
            # fT_all[(c,j), (l,b)] = cc_out[c*20 + (l*4+b), j]: 8 transposing loads
            fT = sp.tile([C, NLVL * BS], F32, tag="fT")
            R = NLVL * BS
            for c in range(N_CORES):
                nc.sync.dma_start(
                    fT[c * CS : (c + 1) * CS, :],
                    cc_out[c * R : (c + 1) * R, :].rearrange("r j -> j r"),
                )

            # ---- matmul 1: hT[(h,l) group] = w1 half.T @ fT level
            psum1 = pp.tile([C, NG * BS], F32, tag="psum1")
            for h in range(2):
                for l in range(NLVL):
                    g = l * 2 + h
                    nc.tensor.matmul(
                        out=psum1[:, g * BS : (g + 1) * BS],
                        lhsT=w1sb[:, l * HID + h * C : l * HID + h * C + C],
                        rhs=fT[:, l * BS : (l + 1) * BS],
                        start=True,
                        stop=True,
                    )

            # ---- train-mode batch norm over the batch (free) axis + ReLU
            p1v = psum1[:].rearrange("p (g b) -> p g b", g=NG)
            sums = sp.tile([C, NG], F32, tag="sums")
            nc.vector.reduce_sum(
                out=sums[:].rearrange("p (g o) -> p g o", o=1),
                in_=p1v,
                axis=mybir.AxisListType.X,
            )
            mean = sp.tile([C, NG], F32, tag="mean")
            nc.vector.tensor_scalar_mul(mean[:], sums[:], 1.0 / BS)
            d = sp.tile([C, NG * BS], F32, tag="d")
            dv = d[:].rearrange("p (g b) -> p g b", g=NG)
            mb = mean[:].rearrange("p (g o) -> p g o", o=1).to_broadcast([C, NG, BS])
            nc.vector.tensor_tensor(out=dv, in0=p1v, in1=mb, op=mybir.AluOpType.subtract)
            sq = sp.tile([C, NG * BS], F32, tag="sq")
            nc.vector.tensor_mul(sq[:], d[:], d[:])
            vs = sp.tile([C, NG], F32, tag="vs")
            nc.vector.reduce_sum(
                out=vs[:].rearrange("p (g o) -> p g o", o=1),
                in_=sq[:].rearrange("p (g b) -> p g b", g=NG),
                axis=mybir.AxisListType.X,
            )
            vpe = sp.tile([C, NG], F32, tag="vpe")
            nc.vector.tensor_scalar(
                out=vpe[:],
                in0=vs[:],
                scalar1=1.0 / BS,
                scalar2=EPS,
                op0=mybir.AluOpType.mult,
                op1=mybir.AluOpType.add,
            )
            std = sp.tile([C, NG], F32, tag="std")
            nc.scalar.activation(std[:], vpe[:], mybir.ActivationFunctionType.Sqrt)
            inv = sp.tile([C, NG], F32, tag="inv")
            nc.vector.reciprocal(inv[:], std[:])
            gsc = sp.tile([C, NG], F32, tag="gsc")
            nc.vector.tensor_mul(gsc[:], inv[:], gsb[:])

            r = sp.tile([C, NG * BS], F32, tag="r")
            rv = r[:].rearrange("p (g b) -> p g b", g=NG)
            gscb = gsc[:].rearrange("p (g o) -> p g o", o=1).to_broadcast([C, NG, BS])
            nc.vector.tensor_tensor(out=rv, in0=dv, in1=gscb, op=mybir.AluOpType.mult)
            bsbb = bsb[:].rearrange("p (g o) -> p g o", o=1).to_broadcast([C, NG, BS])
            nc.vector.tensor_tensor(out=rv, in0=rv, in1=bsbb, op=mybir.AluOpType.add)
            nc.vector.tensor_scalar_max(r[:], r[:], 0.0)

            # ---- matmul 2: out_l = r_l.T @ w2_l (accumulate the two hidden halves)
            psum2 = pp.tile([BS, NLVL * OUT], F32, tag="psum2")
            for l in range(NLVL):
                for h in range(2):
                    g = l * 2 + h
                    nc.tensor.matmul(
                        out=psum2[:, l * OUT : (l + 1) * OUT],
                        lhsT=r[:, g * BS : (g + 1) * BS],
                        rhs=w2sb[:, g * OUT : (g + 1) * OUT],
                        start=(h == 0),
                        stop=(h == 1),
                    )

            osb = sp.tile([BS, NLVL * OUT], F32, tag="osb")
            nc.vector.tensor_add(osb[:], psum2[:], b2sb[:])
            nc.sync.dma_start(
                out.rearrange("l b o -> b l o"),
                osb[:].rearrange("b (l o) -> b l o", l=NLVL),
            )

    nc.compile()
    return nc


def shard_inputs(x0, x1, x2, x3, x4, p, w1, gamma, beta, w2, b2):
    """Build the 8 per-core input maps (numpy only, no index-dependent slicing)."""
    xs = [x0, x1, x2, x3, x4]
    w1t_np = np.ascontiguousarray(np.asarray(w1, dtype=np.float32).transpose(0, 2, 1))
    w2t_np = np.ascontiguousarray(np.asarray(w2, dtype=np.float32).transpose(0, 2, 1))
    gam_np = np.ascontiguousarray(np.asarray(gamma, dtype=np.float32))
    bet_np = np.ascontiguousarray(np.asarray(beta, dtype=np.float32))
    b2r_np = np.ascontiguousarray(
        np.broadcast_to(np.asarray(b2, dtype=np.float32).reshape(1, NLVL * OUT), (BS, NLVL * OUT))
    )
    p32_np = np.ascontiguousarray(np.asarray(p).astype(np.int32))

    in_maps = []
    for c in range(N_CORES):
        m = {"p32": p32_np, "w1t": w1t_np, "w2t": w2t_np, "gam": gam_np,
             "bet": bet_np, "b2r": b2r_np}
        for l, x in enumerate(xs):
            s = SIDES[l]
            shard = np.asarray(x[:, c * CS : (c + 1) * CS], dtype=np.float32)
            m[f"x{l}s"] = np.ascontiguousarray(
                shard.transpose(0, 2, 3, 4, 1).reshape(BS * s**3, CS)
            )
        in_maps.append(m)
    return in_maps


_NC_CACHE = None


def kernel(x0, x1, x2, x3, x4, p, w1, b1, gamma, beta, w2, b2):
    global _NC_CACHE
    if _NC_CACHE is None:
        _NC_CACHE = build_program()
    nc = _NC_CACHE
    in_maps = shard_inputs(x0, x1, x2, x3, x4, p, w1, gamma, beta, w2, b2)
    res = run_bass_kernel_spmd(nc, in_maps, list(range(N_CORES)))
    return np.asarray(res.results[0]["out"], dtype=np.float32)
